# revision 32
# baseline (speedup 1.0000x reference)
"""Trainium2 Bass kernel for multi-head self-attention with RoPE (causal).

Problem shape (hardcoded): x [1, 4096, 1024], 16 heads, d_k=64, fp32.
Sharding: tensor-parallel over heads -- 2 heads per NeuronCore, 8 cores.
Each core computes Q/K/V projections for its 2 heads, RoPE, causal
attention, and a full [4096, 1024] partial of the output projection
(columns of wo matching its heads). Partials are summed on the host
(the all-reduce of row-parallel linear, done at unshard time).

Design (evolved through perfetto-trace-driven iterations from a 308us
phase-split baseline to ~236us):
  - Everything bf16 on device (fp32r matmuls measured 2.7x slower than
    bf16; DMA halves). PSUM accumulation stays fp32.
  - Scores are row-tiled: Q/K live as [128, S] with head0 dims on
    partitions 0-63 and head1 on 64-127. Slicing at base partition
    0/64 auto-derives matmul tile_position, so both heads' d_k=64
    score matmuls run concurrently in the split 64x128 PE array
    (measured dt~3ns between the pair). No zero-padding of K.
  - Scores are computed transposed, ST[k, q]; post-exp P feeds P@V as
    the moving operand. V carries a ones column per head so P@V also
    yields softmax denominators (psum row 64).
  - The kernel is ACT(exp)/PE co-bound (~160us exp stream vs ~175us
    of matmul issue), so everything is software-pipelined around the
    per-chunk score->exp->PV chain:
      * projections land just-in-time ~2 attentions ahead, emitted as
        fillers inside earlier attentions' exp-bound stretches;
      * the output projection of chunk j is deferred into attention
        j+2 so even the big late attentions have PE filler;
      * the first 4 score/exp chunks of attention j+1 are precomputed
        inside attention j's diagonal section (cross-attention
        pipelining), shrinking the ACT hole at chunk boundaries;
      * diagonal masked 128-wide PV spans are split off and deferred
        so the DVE tri-mask mults never block the PE queue head.
  - PV accumulators are released early at chunk boundaries: numerator
    rows evacuate unnormalized (ACT+DVE in parallel) and the 1/d
    multiply runs in SBUF off the critical path.
  - HAM (PE clock gate) management: warm-up matmuls on dummy data
    during the initial DMA wait, and tail matmuls pinned to the last
    exp tiles keep the PE at 2.4 GHz across idle gaps.
  - Input DMA is dependency-ordered on one queue (the two HWDGE
    queues share ~430 GB/s, so splitting hurts the critical path);
    RoPE tables load per-chunk just-in-time.
  - PSUM: 2x [128,2,512] score tiles (4 banks) + 2 P@V accumulators
    + 2 rotating misc banks (projections / swap / V / out-proj) = 8.
"""

import os
import numpy as np

S = 4096
D = 1024
P = 128
DK = 64
SC = 512          # q-chunk width
NQ = S // SC      # 8
NSUB = SC // P    # 4
NKC = S // P      # 32
PO = D // P       # 8 contraction chunks for projections
NCORES = 8
THETA = 10000.0

LAST_EXEC_NS = None
LAST_RESULTS = None

_cache = {}


def _build_bass():
    import concourse.bacc as bacc
    import concourse.tile as tile
    from concourse import mybir

    F32 = mybir.dt.float32
    BF16 = mybir.dt.bfloat16
    EXP = mybir.ActivationFunctionType.Exp
    MULT = mybir.AluOpType.mult
    ADD = mybir.AluOpType.add

    nc = bacc.Bacc("TRN2", target_bir_lowering=False, debug=False)

    xT = nc.dram_tensor("xT", [D, S], BF16, kind="ExternalInput")
    wqT = nc.dram_tensor("wqT", [D, P], BF16, kind="ExternalInput")
    wkT = nc.dram_tensor("wkT", [D, P], BF16, kind="ExternalInput")
    wvT = nc.dram_tensor("wvT", [D, P], BF16, kind="ExternalInput")
    woT = nc.dram_tensor("woT", [P, D], BF16, kind="ExternalInput")
    t1 = nc.dram_tensor("t1", [P, S], BF16, kind="ExternalInput")
    t2s = nc.dram_tensor("t2s", [P, S], BF16, kind="ExternalInput")
    trimask = nc.dram_tensor("trimask", [P, P], BF16, kind="ExternalInput")
    swapmat = nc.dram_tensor("swapmat", [P, P], BF16, kind="ExternalInput")
    out = nc.dram_tensor("out", [S, D], BF16, kind="ExternalOutput")

    xT_t = xT.ap().rearrange("(po pi) s -> pi po s", pi=P)
    wqT_t = wqT.ap().rearrange("(po pi) m -> pi po m", pi=P)
    wkT_t = wkT.ap().rearrange("(po pi) m -> pi po m", pi=P)
    wvT_t = wvT.ap().rearrange("(po pi) m -> pi po m", pi=P)

    with tile.TileContext(nc) as tc:
        with (
            tc.tile_pool(name="persist", bufs=1) as pp,
            tc.tile_pool(name="weights", bufs=1) as wp,
        ):
            # K/V for all chunks persist (causal attention reads history).
            # Head0 dims on partitions 0-63, head1 on 64-127.
            KTr = pp.tile([P, S], BF16, tag="ktr")
            Vp = pp.tile([P, NKC, 256], BF16, tag="vp")

            wq_sb = wp.tile([P, PO, P], BF16, tag="wq")
            wk_sb = wp.tile([P, PO, P], BF16, tag="wk")
            wv_sb = wp.tile([P, PO, P], BF16, tag="wv")
            wo_sb = wp.tile([P, D], BF16, tag="wo")
            t1_sb = wp.tile([P, S], BF16, tag="t1")
            t2_sb = wp.tile([P, S], BF16, tag="t2")
            tri_sb = wp.tile([P, P], BF16, tag="tri")
            swap_sb = wp.tile([P, P], BF16, tag="swap")

            # two HWDGE queues in parallel: compute-critical tensors on
            # the sync queue, RoPE tables on the scalar queue
            nc.sync.dma_start(wq_sb[:], wqT_t)

            # ones columns for the softmax-denominator trick (V padding
            # columns 65-127/193-255 stay uninitialized: they only feed
            # psum partitions 65-127, which are never read)
            ones_sb = wp.tile([P, NKC], F32, tag="ones")
            nc.vector.memset(ones_sb[:], 1.0)
            ones1 = wp.tile([1, DK], F32, tag="ones1")
            nc.vector.memset(ones1[:], 1.0)
            nc.vector.tensor_copy(Vp[:, :, 64], ones_sb[:])
            nc.vector.tensor_copy(Vp[:, :, 192], ones_sb[:])

            with (
                tc.tile_pool(name="spool", bufs=2, space="PSUM") as spool,
                tc.tile_pool(name="psop", bufs=1, space="PSUM") as psop,
                tc.tile_pool(name="misc", bufs=2, space="PSUM") as mpool,
                tc.tile_pool(name="xchunk", bufs=4) as xpool,
                tc.tile_pool(name="qpool", bufs=6) as qpool,
                tc.tile_pool(name="rope", bufs=2) as rpool,
                tc.tile_pool(name="ppool", bufs=10) as ppool,
                tc.tile_pool(name="small", bufs=2) as small,
                tc.tile_pool(name="rbp", bufs=2) as rbp,
                tc.tile_pool(name="otp", bufs=4) as otp,
                tc.tile_pool(name="rtp", bufs=3) as rtp,
            ):
                xts = [None] * NQ
                QTrs = [None] * NQ
                oTts = [None] * NQ

                def emit_xt_dma(jq, eng=None):
                    sl = slice(jq * SC, (jq + 1) * SC)
                    xts[jq] = xpool.tile([P, PO, SC], BF16, tag="xt", name="xtc")
                    (eng or nc.sync).dma_start(xts[jq][:], xT_t[:, :, sl])
                    nc.sync.dma_start(t2_sb[:, sl], t2s.ap()[:, sl])
                    nc.sync.dma_start(t1_sb[:, sl], t1.ap()[:, sl])

                # PE warm-up: throwaway matmuls on garbage data during the
                # input-DMA wait so HAM reaches 2.4 GHz before real work
                dummy = wp.tile([P, SC], BF16, tag="dummy")
                nc.vector.memset(dummy[:], 0.5)
                psd = mpool.tile([P, SC], F32, tag="m", name="psd")
                for _ in range(16):
                    nc.tensor.matmul(psd[:], dummy[:, 0:P], dummy[:],
                                     start=True, stop=True)

                emit_xt_dma(0)
                nc.sync.dma_start(swap_sb[:], swapmat.ap())
                nc.sync.dma_start(wk_sb[:], wkT_t)
                nc.sync.dma_start(wv_sb[:], wvT_t)
                emit_xt_dma(1)
                nc.sync.dma_start(tri_sb[:], trimask.ap())
                nc.sync.dma_start(wo_sb[:], woT.ap())

                def emit_outproj(oTt_p, jq_p, ms, final=False):
                    for m in ms:
                        ssl = slice((jq_p * NSUB + m) * P, (jq_p * NSUB + m + 1) * P)
                        rt = rtp.tile([P, D], BF16, tag="rt")
                        if final and m % 2 == 0:
                            psp = spool.tile([P, 2, SC], F32, tag="st", name="psp")
                            psrs = [psp[:, 0, :], psp[:, 1, :]]
                        else:
                            psrs = [
                                mpool.tile([P, SC], F32, tag="m", name="psr")[:]
                                for _ in range(2)]
                        for jn in range(2):
                            nc.tensor.matmul(
                                psrs[jn], oTt_p[:, m * P:(m + 1) * P],
                                wo_sb[:, jn * SC:(jn + 1) * SC],
                                start=True, stop=True,
                            )
                            with tc.high_priority(1000):
                                if final and jn == 1:
                                    nc.scalar.activation(
                                        rt[:, jn * SC:(jn + 1) * SC], psrs[jn],
                                        mybir.ActivationFunctionType.Copy)
                                else:
                                    nc.vector.tensor_copy(
                                        rt[:, jn * SC:(jn + 1) * SC], psrs[jn])
                        nc.sync.dma_start(out.ap()[ssl, :], rt[:])

                def emit_proj(jq):
                    # ---- projections + RoPE for chunk jq (xt pre-DMA'd) ----
                    sl = slice(jq * SC, (jq + 1) * SC)
                    xt = xts[jq]
                    QTr = qpool.tile([P, SC], BF16, tag="qtr")
                    QTrs[jq] = QTr
                    for w_sb, is_q in ((wq_sb, True), (wk_sb, False)):
                        psq = mpool.tile([P, SC], F32, tag="m", name="psq")[:]
                        pssw = mpool.tile([P, SC], F32, tag="m", name="pssw")[:]
                        for po in range(PO):
                            nc.tensor.matmul(
                                psq, w_sb[:, po, :], xt[:, po, :],
                                start=(po == 0), stop=(po == PO - 1),
                            )
                        # RoPE: dest = t1*psq + swap(t2s*psq)
                        b = rpool.tile([P, SC], BF16, tag="b")
                        nc.vector.tensor_tensor(b[:], t2_sb[:, sl], psq, MULT)
                        nc.tensor.matmul(pssw, swap_sb[:], b[:], start=True, stop=True)
                        a1 = rpool.tile([P, SC], F32, tag="a1")
                        nc.vector.tensor_tensor(a1[:], t1_sb[:, sl], psq, MULT)
                        dest = QTr[:] if is_q else KTr[:, sl]
                        nc.vector.tensor_tensor(dest, a1[:], pssw, ADD)

                    psv = spool.tile([P, 2, SC], F32, tag="st")
                    for m in range(NSUB):
                        kc = jq * NSUB + m
                        for po in range(PO):
                            nc.tensor.matmul(
                                psv[:, 0, m * P:(m + 1) * P],
                                xt[:, po, m * P:(m + 1) * P], wv_sb[:, po, :],
                                start=(po == 0), stop=(po == PO - 1),
                            )
                    for m in range(NSUB):
                        kc = jq * NSUB + m
                        nc.vector.tensor_copy(
                            Vp[:, kc, 0:64], psv[:, 0, m * P:m * P + 64])
                        nc.vector.tensor_copy(
                            Vp[:, kc, 128:192], psv[:, 0, m * P + 64:(m + 1) * P])

                pre_pts = {}

                def emit_pre(jq_next):
                    QTrn = QTrs[jq_next]
                    lst = []
                    for kc in (0, 1, 2, 3):
                        ksl = slice(kc * P, (kc + 1) * P)
                        ps_s = spool.tile([P, 2, SC], F32, tag="st", name="ps_pre")
                        nc.tensor.matmul(
                            ps_s[:, 0, :], KTr[0:DK, ksl], QTrn[0:DK, :],
                            start=True, stop=True,
                        )
                        nc.tensor.matmul(
                            ps_s[:, 1, :], KTr[DK:P, ksl], QTrn[DK:P, :],
                            start=True, stop=True,
                        )
                        pt = ppool.tile([P, 2, SC], BF16, tag="p", name="pt_pre")
                        nc.scalar.activation(pt[:], ps_s[:], EXP, scale=0.125)
                        lst.append((kc, pt))
                    pre_pts[jq_next] = lst

                def emit_att(jq, fillers=None, diag_fillers=(), tail_warm=False):
                    # ---- attention for chunk jq (keys 0..jq) ----
                    # fillers: {full-chunk index -> [callable]} -- independent
                    # PE work (projections of later chunks, out-projections of
                    # earlier ones) dropped into the ACT-bound exp stream
                    QTr = QTrs[jq]
                    pso0 = psop.tile([P, SC], F32, tag="h0")
                    pso1 = psop.tile([P, SC], F32, tag="h1")
                    nfull = NSUB * jq
                    op_after = fillers or {}
                    pre = list(pre_pts.pop(jq, []))
                    # two fresh score/exp chunks first, so the PE queue head
                    # is independent work while the previous chunk's pso
                    # banks drain (the pre-PVs below wait on that release)
                    fresh = []
                    for kc in range(len(pre), min(len(pre) + 2, nfull)):
                        ksl = slice(kc * P, (kc + 1) * P)
                        ps_s = spool.tile([P, 2, SC], F32, tag="st", name="ps_f")
                        nc.tensor.matmul(
                            ps_s[:, 0, :], KTr[0:DK, ksl], QTr[0:DK, :],
                            start=True, stop=True,
                        )
                        nc.tensor.matmul(
                            ps_s[:, 1, :], KTr[DK:P, ksl], QTr[DK:P, :],
                            start=True, stop=True,
                        )
                        pt = ppool.tile([P, 2, SC], BF16, tag="p", name="pt_f")
                        nc.scalar.activation(pt[:], ps_s[:], EXP, scale=0.125)
                        fresh.append((kc, pt))
                    pre += fresh
                    for pos in (fillers or {}):
                        assert pos >= len(pre), f"filler at {pos} dropped (pre={len(pre)})"
                    started = bool(pre) or nfull == 0
                    for kc, pt in pre:
                        nc.tensor.matmul(
                            pso0[:], Vp[:, kc, 0:128], pt[:, 0, :],
                            start=(kc == 0), stop=False,
                        )
                        nc.tensor.matmul(
                            pso1[:], Vp[:, kc, 128:256], pt[:, 1, :],
                            start=(kc == 0), stop=False,
                        )
                    for kc in range(len(pre), nfull):
                        ksl = slice(kc * P, (kc + 1) * P)
                        ps_s = spool.tile([P, 2, SC], F32, tag="st")
                        nc.tensor.matmul(
                            ps_s[:, 0, :], KTr[0:DK, ksl], QTr[0:DK, :],
                            start=True, stop=True,
                        )
                        nc.tensor.matmul(
                            ps_s[:, 1, :], KTr[DK:P, ksl], QTr[DK:P, :],
                            start=True, stop=True,
                        )
                        pt = ppool.tile([P, 2, SC], BF16, tag="p")
                        nc.scalar.activation(pt[:], ps_s[:], EXP, scale=0.125)
                        nc.tensor.matmul(
                            pso0[:], Vp[:, kc, 0:128], pt[:, 0, :],
                            start=(not started), stop=False,
                        )
                        nc.tensor.matmul(
                            pso1[:], Vp[:, kc, 128:256], pt[:, 1, :],
                            start=(not started), stop=False,
                        )
                        started = True
                        for fn in op_after.get(kc, ()):
                            fn()
                    # diagonal straddle chunks: unmasked column span first;
                    # the 128-wide masked spans (which wait on the DVE
                    # tri-mult) are deferred to the end so they never block
                    # the PE queue head.
                    masked = []
                    for t in range(NSUB):
                        kc = nfull + t
                        ksl = slice(kc * P, (kc + 1) * P)
                        col0 = t * P
                        ps_s = spool.tile([P, 2, SC], F32, tag="st")
                        nc.tensor.matmul(
                            ps_s[:, 0, col0:SC], KTr[0:DK, ksl],
                            QTr[0:DK, col0:SC], start=True, stop=True,
                        )
                        nc.tensor.matmul(
                            ps_s[:, 1, col0:SC], KTr[DK:P, ksl],
                            QTr[DK:P, col0:SC], start=True, stop=True,
                        )
                        pt = ppool.tile([P, 2, SC], BF16, tag="p")
                        nc.scalar.activation(
                            pt[:, :, col0:SC], ps_s[:, :, col0:SC], EXP, scale=0.125
                        )
                        with tc.high_priority(2000):
                            nc.vector.tensor_tensor(
                                pt[:, 0, col0:col0 + P], pt[:, 0, col0:col0 + P],
                                tri_sb[:], MULT,
                            )
                            nc.vector.tensor_tensor(
                                pt[:, 1, col0:col0 + P], pt[:, 1, col0:col0 + P],
                                tri_sb[:], MULT,
                            )
                        first = (nfull == 0 and t == 0)
                        if col0 + P < SC:
                            nc.tensor.matmul(
                                pso0[:, col0 + P:SC], Vp[:, kc, 0:128],
                                pt[:, 0, col0 + P:SC], start=first, stop=False,
                            )
                            nc.tensor.matmul(
                                pso1[:, col0 + P:SC], Vp[:, kc, 128:256],
                                pt[:, 1, col0 + P:SC], start=first, stop=False,
                            )
                            first = False
                        masked.append((kc, col0, pt, first))
                    # independent PE filler lands here: its matmuls cover the
                    # masked-PV tri-mult latency, and its DVE work queues
                    # behind the tri-mults so it never delays them
                    for fn in diag_fillers:
                        fn()
                    for i, (kc, col0, pt, first) in enumerate(masked):
                        last = (i == len(masked) - 1)
                        nc.tensor.matmul(
                            pso0[:, col0:col0 + P], Vp[:, kc, 0:128],
                            pt[:, 0, col0:col0 + P], start=first, stop=last,
                        )
                        nc.tensor.matmul(
                            pso1[:, col0:col0 + P], Vp[:, kc, 128:256],
                            pt[:, 1, col0:col0 + P], start=first, stop=last,
                        )

                    if tail_warm:
                        # keep HAM at 2.4 GHz through the final normalize gap;
                        # read the last diag exps' pt tiles so the scheduler
                        # cannot hoist these out of the tail window
                        psd2 = mpool.tile([P, SC], F32, tag="m", name="psd2")[:]
                        ptw2 = masked[-2][2]
                        ptw3 = masked[-1][2]
                        for r in range(16):
                            ptw = ptw2 if r % 2 == 0 else ptw3
                            c0 = 2 * P if r % 2 == 0 else 3 * P
                            nfree = 2 * (SC - c0)
                            nc.tensor.matmul(
                                psd2[0:P, 0:nfree], ptw[:, 0, c0:c0 + P],
                                ptw[:, :, c0:SC], start=True, stop=True)
                    # ---- normalize: oT[h] = pso_h[0:64] / pso_h[64] ----
                    # pso banks are released early: the numerator rows are
                    # evacuated unnormalized via ACT while DVE grabs the
                    # denominator rows; the reciprocal multiply then runs
                    # purely in SBUF, off the PV-accumulator critical path.
                    oTt = otp.tile([P, SC], BF16, tag="ot")
                    dn0 = small.tile([1, SC], F32, tag="dn")
                    nc.vector.tensor_copy(dn0[:], pso0[64:65, :])
                    dn1 = small.tile([1, SC], F32, tag="dn")
                    nc.scalar.activation(
                        dn1[:], pso1[64:65, :], mybir.ActivationFunctionType.Copy)
                    oU0 = rbp.tile([DK, SC], BF16, tag="ou")
                    nc.scalar.activation(
                        oU0[:], pso0[0:DK, :], mybir.ActivationFunctionType.Copy)
                    oU1 = rbp.tile([DK, SC], BF16, tag="ou")
                    nc.vector.tensor_copy(oU1[:], pso1[0:DK, :])
                    rr0 = small.tile([1, SC], F32, tag="rr")
                    nc.vector.reciprocal_approx_fast(rr0[:], dn0[:])
                    rr1 = small.tile([1, SC], F32, tag="rr")
                    nc.vector.reciprocal_approx_fast(rr1[:], dn1[:])
                    if tail_warm:
                        # PE is idle at the tail: broadcast 1/d with a rank-1
                        # fp32 matmul instead of the slow gpsimd path
                        rbp0 = mpool.tile([P, SC], F32, tag="m", name="rbp0")
                        rbp1 = mpool.tile([P, SC], F32, tag="m", name="rbp1")
                        nc.tensor.matmul(rbp0[0:DK, :], ones1[:], rr0[:],
                                         start=True, stop=True)
                        nc.tensor.matmul(rbp1[0:DK, :], ones1[:], rr1[:],
                                         start=True, stop=True)
                        nc.vector.tensor_tensor(
                            oTt[0:DK, :], oU0[:], rbp0[0:DK, :], MULT)
                        nc.vector.tensor_tensor(
                            oTt[DK:P, :], oU1[:], rbp1[0:DK, :], MULT)
                    else:
                        rb0 = rbp.tile([DK, SC], F32, tag="rb")
                        nc.gpsimd.partition_broadcast(rb0[:], rr0[:])
                        rb1 = rbp.tile([DK, SC], F32, tag="rb")
                        nc.gpsimd.partition_broadcast(rb1[:], rr1[:])
                        nc.vector.tensor_tensor(oTt[0:DK, :], oU0[:], rb0[:], MULT)
                        nc.vector.tensor_tensor(oTt[DK:P, :], oU1[:], rb1[:], MULT)
                    oTts[jq] = oTt

                def op(jq_p, mi):
                    return lambda: emit_outproj(oTts[jq_p], jq_p, (mi,))

                def pj(jq_p):
                    return lambda: emit_proj(jq_p)

                # Just-in-time interleave: proj(jq) lands ~2 attentions ahead
                # of att(jq); outproj(jq) is deferred ~2 attentions so even
                # the big late attentions have independent PE filler work.
                emit_proj(0)
                emit_proj(1)
                emit_xt_dma(2)
                emit_att(0, diag_fillers=(lambda: emit_pre(1),))
                emit_proj(2)
                emit_xt_dma(3)
                emit_att(1, {}, diag_fillers=(lambda: emit_pre(2), pj(3)))
                emit_xt_dma(4)
                emit_att(2, {6: [op(0, 0), op(0, 1)],
                             7: [op(0, 2), op(0, 3)]},
                         diag_fillers=(lambda: emit_pre(3), pj(4)))
                emit_xt_dma(5)
                emit_att(3, {6: [op(1, 0)], 7: [op(1, 1)],
                             8: [op(1, 2)], 9: [op(1, 3)]},
                         diag_fillers=(lambda: emit_pre(4), pj(5)))
                emit_xt_dma(6)
                emit_att(4, {6: [op(2, 0)], 7: [op(2, 1)],
                             8: [op(2, 2)], 9: [op(2, 3)]},
                         diag_fillers=(lambda: emit_pre(5), pj(6)))
                emit_xt_dma(7)
                emit_att(5, {6: [op(3, 0)], 7: [op(3, 1)],
                             8: [op(3, 2)], 9: [op(3, 3)]},
                         diag_fillers=(lambda: emit_pre(6), pj(7)))
                emit_att(6, {6: [op(4, 0)], 7: [op(4, 1)],
                             8: [op(4, 2)]},
                         diag_fillers=(lambda: emit_pre(7), op(4, 3)))
                emit_att(7, {9: [op(5, 0)], 11: [op(5, 1)],
                             13: [op(5, 2)], 15: [op(5, 3)],
                             17: [op(6, 0)], 19: [op(6, 1)]},
                         diag_fillers=(op(6, 2), op(6, 3)),
                         tail_warm=True)
                emit_outproj(oTts[7], 7, range(NSUB), final=True)

    nc.compile()
    return nc


def _rope_tables():
    inv_freq = 1.0 / (THETA ** (np.arange(0, DK, 2, dtype=np.float64) / DK))  # [32]
    pos = np.arange(S, dtype=np.float64)
    freqs = pos[:, None] * inv_freq[None, :]      # [S, 32]
    cosT = np.cos(freqs).T.astype(np.float32)     # [32, S]
    sinT = np.sin(freqs).T.astype(np.float32)
    # t1 rows (per 64-block): [cos; cos];   t2s rows: [+sin; -sin]
    t1 = np.tile(np.concatenate([cosT, cosT], axis=0), (2, 1))      # [128, S]
    t2s_ = np.tile(np.concatenate([sinT, -sinT], axis=0), (2, 1))   # [128, S]
    return np.ascontiguousarray(t1), np.ascontiguousarray(t2s_)


def _host_prep(x, wq, wk, wv, wo):
    from ml_dtypes import bfloat16

    x2 = np.asarray(x, dtype=np.float32).reshape(S, D)
    xT = np.ascontiguousarray(x2.T).astype(bfloat16)

    # even/odd de-interleave permutation within each head's 64 rows
    perm64 = np.concatenate([np.arange(0, DK, 2), np.arange(1, DK, 2)])
    perm128 = np.concatenate([perm64, perm64 + DK])

    t1, t2s_ = _rope_tables()
    t1 = t1.astype(bfloat16)
    t2s_ = t2s_.astype(bfloat16)
    trimask = np.triu(np.ones((P, P), dtype=np.float32)).astype(bfloat16)
    swp = np.zeros((P, P), dtype=np.float32)
    for b in range(2):
        for i in range(32):
            swp[b * 64 + i, b * 64 + 32 + i] = 1.0
            swp[b * 64 + 32 + i, b * 64 + i] = 1.0
    swp = swp.astype(bfloat16)

    wq = np.asarray(wq, dtype=np.float32)
    wk = np.asarray(wk, dtype=np.float32)
    wv = np.asarray(wv, dtype=np.float32)
    wo = np.asarray(wo, dtype=np.float32)

    in_maps = []
    for c in range(NCORES):
        rows = slice(P * c, P * (c + 1))
        wq_c = wq[rows][perm128]
        wk_c = wk[rows][perm128]
        in_maps.append({
            "xT": xT,
            "wqT": np.ascontiguousarray(wq_c.T).astype(bfloat16),
            "wkT": np.ascontiguousarray(wk_c.T).astype(bfloat16),
            "wvT": np.ascontiguousarray(wv[rows].T).astype(bfloat16),
            "woT": np.ascontiguousarray(wo[:, rows].T).astype(bfloat16),
            "t1": t1,
            "t2s": t2s_,
            "trimask": trimask,
            "swapmat": swp,
        })
    return in_maps


def _install_ntff_hook():
    """Register the axon NTFF profiling hook (missing antenv.axon_hooks shim)."""
    import sys
    import types
    import importlib

    try:
        import antenv.axon_hooks  # noqa: F401
        return
    except ImportError:
        pass
    try:
        import antenv
        boot = importlib.import_module("trn_agent_boot.trn_boot")
        mod = types.ModuleType("antenv.axon_hooks")
        state = {"hook": None}
        mod.set_axon_ntff_profile_hook = lambda h: state.update(hook=h)
        mod.get_axon_ntff_profile_hook = lambda: state["hook"]
        sys.modules["antenv.axon_hooks"] = mod
        antenv.axon_hooks = mod
        hook = boot._ntff_profile_via_ctypes("/opt/axon/libaxon_pjrt.so")
        mod.set_axon_ntff_profile_hook(hook)
    except Exception as e:  # profiling is best-effort
        print(f"ntff hook install failed: {e}")


def kernel(x, wq, wk, wv, wo):
    global LAST_EXEC_NS, LAST_RESULTS
    from concourse import bass_utils

    trace_requested = bool(int(os.environ.get("TRN_TRACE", "0")))
    if trace_requested:
        _install_ntff_hook()
        # artifact upload needs remote storage; stub it out in this sandbox
        bass_utils.upload_artifacts = lambda tmpdir: "local://" + str(tmpdir)

    if "nc" not in _cache:
        _cache["nc"] = _build_bass()
    nc = _cache["nc"]

    in_maps = _host_prep(x, wq, wk, wv, wo)
    res = bass_utils.run_bass_kernel_spmd(
        nc, in_maps, core_ids=list(range(NCORES)), trace=trace_requested
    )
    LAST_EXEC_NS = res.exec_time_ns
    LAST_RESULTS = res
    acc = np.zeros((S, D), dtype=np.float32)
    for r in res.results:
        acc += np.asarray(r["out"], dtype=np.float32)
    return acc.reshape(1, S, D)
